# revision 24
# baseline (speedup 1.0000x reference)
"""Trainium2 Bass kernel for MoE routing (2-layer expert MLP + softmax).

Strategy: expert-parallel across the 8 NeuronCores. The reference computes
all 8 experts for every sample and then gathers the one selected by
`domain`; mathematically only the selected expert's MLP matters per sample.
The host groups samples by expert, core e receives only the ~B/8 samples
routed to expert e (padded to a uniform per-core capacity so all cores run
the same SPMD program) plus expert e's weights. Each core runs a dense
2-layer MLP + softmax in a transposed layout:

    hT[f2, n]  = relu(W1[:, f2].T @ xT[:, n] + b1[f2])   (PE + ACT)
    lT[c, n]   = W2[:, c].T @ hT[:, n]                   (PE)
    expT       = exp(lT + b2)                            (ACT)
    sT[c, n]   = ones[C,C].T @ expT                      (PE partition sum,
                                                          pre-broadcast to C)
    out[c, n]  = expT * (1 / sT)                         (DVE)

All matmul operands are bfloat16 (PSUM accumulation stays fp32): bf16 runs
at the same 1 cycle/row PE rate as float32r but halves every DMA stream,
SBUF footprint, and the startup fill. Measured end-to-end relative error
is ~6e-3 against the fp32 reference (gate: 2e-2).

DMA discipline (measured on HW): a consumer's semaphore wait covers every
DMA emitted on that queue up to the consumer's emission point, and each
HWDGE queue only holds ~4 outstanding descriptors-jobs — so batching DMAs
up front makes the first matmul wait for the WHOLE stream (the v2 kernel
lost 10us to exactly that). Therefore every DMA is emitted immediately
before the compute that consumes it: w1 m-blocks interleave with chunk 0's
m-waves, and chunk ci+1's x load is emitted at the END of chunk ci's waves
(emitting it at the head would make chunk ci's own matmuls wait for it;
the queues still stream back-to-back because the sequencer runs ahead of
the PE). Chunk loads are split in halves across the sync and gpsimd rings.

Schedule (per core): the batch is cut into <=512-wide chunks (PSUM bank
width, capacity = exact max per-expert count — no 128-padding), processed
chunk-outer (only ~5 PSUM banks live). Stationary operands are zero-padded
to 128 output columns (matmuls with 64-wide output or contraction run
~1.5x slower). Layer-2 + softmax stages for chunk c are emitted
interleaved between the m-waves of chunk c+1, giving every cross-engine
dependency a full m-wave of slack. A warm-up burst of dummy matmuls (with
a fine-grained 128-row tail) bridges the PE p-state ramp into the first
real wave — on TRN2 any PE idle gap drops the clock back to ~1GHz for the
next ~3us of execution, so the stream must start hot and stay dense.
"""

import math
from collections import deque

import ml_dtypes
import numpy as np

import concourse.bacc as bacc
import concourse.bass as bass
import concourse.mybir as mybir
import concourse.tile as tile
from concourse.bass import ds
from concourse.bass_utils import run_bass_kernel_spmd

N_CORES = 8
BF16 = ml_dtypes.bfloat16

_program_cache: dict[tuple, object] = {}


def _chunk_sizes(cap: int) -> list[int]:
    """<=512-wide chunks (PSUM bank width)."""
    q, r = divmod(cap, 512)
    return [512] * q + ([r] if r else [])


def _build_program(cap: int, F1: int, F2: int, C: int):
    key = (cap, F1, F2, C)
    if key in _program_cache:
        return _program_cache[key]

    assert F1 % 128 == 0 and F2 % 128 == 0
    K1 = F1 // 128
    M1 = F2 // 128
    K2 = F2 // 128
    assert C <= 128

    f32 = mybir.dt.float32
    bf16 = mybir.dt.bfloat16
    nc = bacc.Bacc(None, target_bir_lowering=False, debug=False)

    chunks = _chunk_sizes(cap)
    offs = [0]
    for cn in chunks:
        offs.append(offs[-1] + cn)

    x_d = [
        nc.dram_tensor(f"xt{ci}", [128, K1, cn], bf16, kind="ExternalInput")
        for ci, cn in enumerate(chunks)
    ]
    w1_d = nc.dram_tensor("w1", [128, M1, K1, 128], bf16, kind="ExternalInput")
    b1_d = nc.dram_tensor("b1t", [128, M1], f32, kind="ExternalInput")
    w2_d = nc.dram_tensor("w2", [128, K2, 128], bf16, kind="ExternalInput")
    b2_d = nc.dram_tensor("b2t", [C, 1], f32, kind="ExternalInput")
    out_d = nc.dram_tensor("outT", [C, cap], bf16, kind="ExternalOutput")

    with tile.TileContext(nc) as tc:
        with (
            tc.tile_pool(name="const", bufs=1) as const_pool,
            # bufs=2 doubles as a prefetch throttle: chunk c+2's DMA trigger
            # carries a WAR wait on chunk c's last matmul, so only chunk 0,
            # chunk 1 and the weights compete for the startup fill; later
            # chunks stream consumption-paced, always one chunk ahead.
            tc.tile_pool(name="xin", bufs=2) as x_pool,
            tc.tile_pool(name="h", bufs=2 * M1 + 2) as h_pool,
            tc.tile_pool(name="exp", bufs=2) as e_pool,
            tc.tile_pool(name="out", bufs=2) as o_pool,
            tc.tile_pool(name="rec", bufs=2) as r_pool,
            tc.tile_pool(name="ph", bufs=5, space="PSUM") as ph_pool,
            tc.tile_pool(name="pl", bufs=2, space="PSUM") as pl_pool,
            tc.tile_pool(name="pb", bufs=1, space="PSUM") as pb_pool,
        ):
            # Scalar (ACT) ring: biases then the first w1 m-block — only
            # what the first m-wave needs; later m-blocks are emitted
            # between chunk-0 waves.
            b1_sb = const_pool.tile([128, M1], f32)
            nc.scalar.dma_start(b1_sb[:], b1_d[:])
            b2_sb = const_pool.tile([C, 1], f32)
            nc.scalar.dma_start(b2_sb[:], b2_d[:])
            # Split the first weight block (and chunk 0 below) into two DMA
            # jobs each: the HWDGE pipelines ~4 jobs per queue, so splitting
            # gives the critical startup transfers a larger share of the
            # descriptor pipeline against the prefetch streams behind them.
            w1_sb = const_pool.tile([128, M1, K1, 128], bf16)
            nc.scalar.dma_start(
                w1_sb[:, 0, : K1 // 2, :], w1_d[:, 0, : K1 // 2, :]
            )
            nc.scalar.dma_start(
                w1_sb[:, 0, K1 // 2 :, :], w1_d[:, 0, K1 // 2 :, :]
            )
            # w2 padded to 128 output columns: matmuls with only 64 output
            # partitions measure ~333ns/512 rows vs 216ns at 128 partitions,
            # so layer 2 runs on a zero-padded [*, 128] stationary tile.
            w2_sb = const_pool.tile([128, K2, 128], bf16)

            # Warm-up operand memset FIRST on the gpsimd queue (before
            # any DMA) so the warm-up starts as soon as the engine boots.
            wu_x = const_pool.tile([128, 512], bf16)
            nc.gpsimd.memset(wu_x[:], 0.0)
            # ones[128, 128]: all-ones stationary with both the
            # contraction and output dims padded to 128 (narrow matmuls are
            # ~1.5x slower; see w2 padding note). The exp tile's partitions
            # C..127 are zeroed so the padded contraction adds nothing.
            ones_cc = const_pool.tile([128, 128], bf16)
            nc.gpsimd.memset(ones_cc[:], 1.0)

            # Sync+gpsimd rings: chunk 0 x halves — its consumers must not
            # wait on anything queued later.
            xt = []
            t = x_pool.tile([128, K1, chunks[0]], bf16, tag="xt", name="xt0")
            nc.sync.dma_start(t[:, : K1 // 2, :], x_d[0][:, : K1 // 2, :])
            nc.gpsimd.dma_start(t[:, K1 // 2 :, :], x_d[0][:, K1 // 2 :, :])
            xt.append(t)

            # Warm-up: ramp the PE p-state while the startup fill (w1 + the
            # first x chunks, ~2.5MB -> ready at ~13.5us) lands. Sized to
            # end right as the fill completes so the real stream follows
            # hot with no idle gap (any PE gap drops the clock to ~1GHz
            # for the next ~3us). The 128-row tail keeps the handoff
            # fine-grained.
            for i in range(6):
                wu_ps = ph_pool.tile([128, 512], f32, tag="ph", name=f"wu{i}")
                nc.tensor.matmul(
                    wu_ps[:], wu_x[:, :128], wu_x[:], start=True, stop=True
                )
            for i in range(55):
                wu_ps = ph_pool.tile([128, 128], f32, tag="ph", name=f"wv{i}")
                nc.tensor.matmul(
                    wu_ps[:], wu_x[:, :128], wu_x[:, :128], start=True, stop=True
                )

            stages: deque = deque()

            def stage_l2(ci: int, cn: int, ht: list):
                pl = pl_pool.tile([128, cn], f32, tag="pl")
                for k in range(K2):
                    nc.tensor.matmul(
                        pl[:],
                        w2_sb[:, k, :],
                        ht[k][:],
                        start=(k == 0),
                        stop=(k == K2 - 1),
                    )
                expt = e_pool.tile([128, cn], bf16, tag="expt")
                nc.gpsimd.memset(expt[C:128, :], 0.0)
                nc.scalar.activation(
                    expt[0:C, :],
                    pl[0:C, :],
                    mybir.ActivationFunctionType.Exp,
                    bias=b2_sb[:, 0:1],
                )
                stages.append(lambda: stage_norm(ci, cn, expt))

            def stage_norm(ci: int, cn: int, expt):
                pb = pb_pool.tile([128, cn], f32, tag="pb")
                nc.tensor.matmul(pb[:], ones_cc[:], expt[:], start=True, stop=True)
                rec = r_pool.tile([C, cn], f32, tag="rec")
                nc.vector.reciprocal_approx_fast(rec[:], pb[0:C, :])
                ot = o_pool.tile([C, cn], bf16, tag="ot")
                nc.vector.tensor_mul(ot[:], expt[0:C, :], rec[:])
                # Sync HWDGE, not gpsimd SWDGE: SWDGE descriptor generation
                # costs ~1us and the final store sits on the critical tail.
                nc.sync.dma_start(out_d[:, ds(offs[ci], cn)], ot[:])

            for ci, cn in enumerate(chunks):
                ht = []
                for m in range(M1):
                    ph = ph_pool.tile([128, cn], f32, tag="ph")
                    for k in range(K1):
                        nc.tensor.matmul(
                            ph[:],
                            w1_sb[:, m, k, :],
                            xt[ci][:, k, :],
                            start=(k == 0),
                            stop=(k == K1 - 1),
                        )
                    hm = h_pool.tile([128, cn], bf16, tag="ht")
                    nc.scalar.activation(
                        hm[:],
                        ph[:],
                        mybir.ActivationFunctionType.Relu,
                        bias=b1_sb[:, ds(m, 1)],
                    )
                    ht.append(hm)
                    if ci == 0 and m + 1 < M1:
                        # Emitted AFTER wave m so wave m+1 (not wave m)
                        # carries the wait for this transfer.
                        nc.scalar.dma_start(
                            w1_sb[:, m + 1, :, :], w1_d[:, m + 1, :, :]
                        )
                    if ci == 0 and m == 2:
                        nc.scalar.dma_start(w2_sb[:], w2_d[:])
                    if stages:
                        stages.popleft()()
                # Prefetch next chunk's x AFTER this chunk's waves: a
                # consumer waits on every DMA emitted earlier on the same
                # queue, so emitting the prefetch first would make THIS
                # chunk's matmuls wait for the NEXT chunk's transfer (cost
                # ~4us at startup). The sequencer still issues it
                # back-to-back behind the previous loads. Halves go on
                # sync+gpsimd for twice the early descriptor share.
                if ci + 1 < len(chunks):
                    cnn = chunks[ci + 1]
                    t = x_pool.tile(
                        [128, K1, cnn], bf16, tag="xt", name=f"xt{ci + 1}"
                    )
                    # Gate the prefetch's ISSUE (not just its consumers)
                    # behind the current chunk's data: this corner copy
                    # reads chunk ci (both DMA halves) and writes a corner
                    # of chunk ci+1's tile, so the WAW dependency stops the
                    # next D2D from interleaving its descriptors with the
                    # in-flight transfer and halving its bandwidth. The DMA
                    # overwrites the corner immediately.
                    nc.vector.tensor_copy(
                        t[0:1, :, 0:1], xt[ci][0:1, :, 0:1]
                    )
                    nc.sync.dma_start(
                        t[:, : K1 // 2, :], x_d[ci + 1][:, : K1 // 2, :]
                    )
                    nc.gpsimd.dma_start(
                        t[:, K1 // 2 :, :], x_d[ci + 1][:, K1 // 2 :, :]
                    )
                    xt.append(t)
                stages.append(lambda ci=ci, cn=cn, ht=ht: stage_l2(ci, cn, ht))
            while stages:
                stages.popleft()()

    nc.compile()
    _program_cache[key] = nc
    return nc


def _pad_w2(w2e, K2, C):
    """[F2, C] -> [128, K2, 128] with zero-padded output columns."""
    p = np.zeros((128, K2, 128), BF16)
    p[:, :, :C] = w2e.reshape(K2, 128, C).transpose(1, 0, 2)
    return p


def kernel(domain, x, W1, b1, W2, b2):
    domain = np.asarray(domain)
    x = np.ascontiguousarray(np.asarray(x, dtype=np.float32))
    W1 = np.asarray(W1, dtype=np.float32)
    b1 = np.asarray(b1, dtype=np.float32)
    W2 = np.asarray(W2, dtype=np.float32)
    b2 = np.asarray(b2, dtype=np.float32)

    B, F1 = x.shape
    E, _, F2 = W1.shape
    C = W2.shape[2]
    K1 = F1 // 128
    K2 = F2 // 128
    M1 = F2 // 128
    assert E == N_CORES

    xb = x.astype(BF16)
    W1b = W1.astype(BF16)
    W2b = W2.astype(BF16)

    idx = [np.nonzero(domain == e)[0] for e in range(E)]
    counts = [len(i) for i in idx]
    cap = max(512, max(counts))
    chunks = _chunk_sizes(cap)

    nc = _build_program(cap, F1, F2, C)

    in_maps = []
    for e in range(E):
        xT = np.zeros((F1, cap), BF16)
        xT[:, : counts[e]] = xb[idx[e]].T
        # [F1, cap] -> [128, K1, cap] SBUF tile layout.
        xT4 = xT.reshape(K1, 128, cap).transpose(1, 0, 2)
        m = {
            "w1": np.ascontiguousarray(
                W1b[e].reshape(K1, 128, M1, 128).transpose(1, 2, 0, 3)
            ),
            "b1t": np.ascontiguousarray(b1[e].reshape(M1, 128).T),
            "w2": _pad_w2(W2b[e], K2, C),
            "b2t": np.ascontiguousarray(b2[e].reshape(C, 1)),
        }
        n0 = 0
        for ci, cn in enumerate(chunks):
            m[f"xt{ci}"] = np.ascontiguousarray(xT4[:, :, n0 : n0 + cn])
            n0 += cn
        in_maps.append(m)

    res = run_bass_kernel_spmd(nc, in_maps, core_ids=list(range(N_CORES)))

    out = np.empty((B, C), np.float32)
    for e in range(E):
        out[idx[e]] = res.results[e]["outT"][:, : counts[e]].T.astype(np.float32)
    return out
